# revision 1
# baseline (speedup 1.0000x reference)
"""DeeperSet aggregation kernel for 8 Trainium2 NeuronCores.

Strategy: data-parallel over contiguous graph-id ranges (2048 graphs/core).
Segment boundaries are host-known (batch is an input), so segment-sum and
the xg[batch] gather are expressed as matmuls against host-built one-hot
tiles.  LayerNorm (gamma=1, beta=0, biases=0 in this model) reduces to a
per-node positive scale r = 1/sqrt(mean(u^2)+eps) with mean-centering folded
into the weights on the host; r commutes through ReLU and the second linear,
so it is applied inside the ReLU activation (ACT per-partition scale).
"""

import sys

sys.path.insert(0, "/opt/trn_rl_repo")

import numpy as np

import concourse.bass as bass
import concourse.tile as tile
from concourse import bacc, mybir
from concourse.bass_utils import run_bass_kernel_spmd
from concourse.masks import make_identity

F32 = mybir.dt.float32
F16 = mybir.dt.float16
ALU = mybir.AluOpType
ACTF = mybir.ActivationFunctionType

LN_EPS = 1e-5
NCORES = 8
SPB = 128          # segments (graphs) per block
T = 128            # nodes per tile
SB = 16            # tiles per stats batch
DB = 8             # tiles per DMA batch


def _center(w, g):
    return ((w - w.mean(axis=1, keepdims=True)) * g[None, :]).astype(np.float32)


def _prep_host(inputs):
    x = np.asarray(inputs["x"], np.float32)
    y = np.asarray(inputs["y"], np.float32)
    batch = np.asarray(inputs["batch"], np.int64)
    N, E = x.shape
    B, YD = y.shape
    H = inputs["l0_lw1"].shape[1]

    for k in ("l0_lb1", "l0_lbt", "l0_lb2", "l0_gb1", "l0_gbt", "l0_gb2",
              "lr_lb1", "lr_lbt", "lr_lb2", "lr_gb1", "lr_gbt", "lr_gb2", "cb"):
        assert np.abs(np.asarray(inputs[k])).max() < 1e-12, f"{k} must be zero"
    for k in ("l0_lg", "l0_gg", "lr_lg", "lr_gg"):
        assert np.abs(np.asarray(inputs[k]) - 1.0).max() < 1e-12, f"{k} must be one"

    B_LOC = B // NCORES
    NBLK = B_LOC // SPB
    nblocks = B // SPB
    edges = np.searchsorted(batch, np.arange(0, B + 1, SPB)).astype(np.int64)
    cnts = np.diff(edges)
    maxblk = int(np.ceil(cnts.max() / T)) if N > 0 else 1
    MAXBLK = max(SB, ((maxblk + SB - 1) // SB) * SB)
    NT = NBLK * MAXBLK          # tiles per core
    NPADC = NT * T              # padded nodes per core

    xT = [np.zeros((E, NPADC), np.float16) for _ in range(NCORES)]
    OT = [np.zeros((NT // DB, T, DB, SPB), np.float16) for _ in range(NCORES)]
    OG = [np.zeros((NT // DB, SPB, DB, T), np.float16) for _ in range(NCORES)]
    ysT = [None] * NCORES
    for c in range(NCORES):
        for k in range(NBLK):
            j = c * NBLK + k
            n0, n1 = int(edges[j]), int(edges[j + 1])
            cnt = n1 - n0
            if cnt == 0:
                continue
            base = k * MAXBLK * T
            xT[c][:, base:base + cnt] = x[n0:n1].T.astype(np.float16)
            a = base + np.arange(cnt)
            t = a // T
            p = a % T
            g = (batch[n0:n1] - j * SPB).astype(np.int64)
            OT[c][t // DB, p, t % DB, g] = 1.0
            OG[c][t // DB, g, t % DB, p] = 1.0
        ysT[c] = np.ascontiguousarray(y[c * B_LOC:(c + 1) * B_LOC].T).astype(np.float16)

    f16 = lambda w: np.ascontiguousarray(w).astype(np.float16)
    l0_w1f = _center(np.asarray(inputs["l0_lw1"], np.float32), np.asarray(inputs["l0_lg"], np.float32))
    W1X, W1G = [f16(l0_w1f)], [None]
    W2 = [f16(np.asarray(inputs["l0_lw2"], np.float32))]
    GW1 = [f16(_center(np.asarray(inputs["l0_gw1"], np.float32), np.asarray(inputs["l0_gg"], np.float32)))]
    GW2 = [f16(np.asarray(inputs["l0_gw2"], np.float32))]
    for i in range(2):
        w1f = _center(np.asarray(inputs["lr_lw1"][i], np.float32), np.asarray(inputs["lr_lg"][i], np.float32))
        W1X.append(f16(w1f[:E]))
        W1G.append(f16(w1f[E:]))
        W2.append(f16(np.asarray(inputs["lr_lw2"][i], np.float32)))
        GW1.append(f16(_center(np.asarray(inputs["lr_gw1"][i], np.float32), np.asarray(inputs["lr_gg"][i], np.float32))))
        GW2.append(f16(np.asarray(inputs["lr_gw2"][i], np.float32)))
    CW = f16(np.asarray(inputs["cw"], np.float32))

    geom = dict(N=N, E=E, B=B, YD=YD, H=H, B_LOC=B_LOC, NBLK=NBLK,
                MAXBLK=MAXBLK, NT=NT, NPADC=NPADC)
    shared = dict(CW=CW)
    for l in range(3):
        shared[f"W1X{l}"] = W1X[l]
        shared[f"W2_{l}"] = W2[l]
        shared[f"GW1_{l}"] = GW1[l]
        shared[f"GW2_{l}"] = GW2[l]
        if l > 0:
            shared[f"W1G{l}"] = W1G[l]
    percore = [dict(xT=xT[c], OT=OT[c], OG=OG[c], ysT=ysT[c]) for c in range(NCORES)]
    return geom, shared, percore


def _build_program(geom):
    E, H, YD = geom["E"], geom["H"], geom["YD"]
    B_LOC, NBLK, MAXBLK, NT, NPADC = (geom["B_LOC"], geom["NBLK"],
                                      geom["MAXBLK"], geom["NT"], geom["NPADC"])
    HC = H // 128  # H chunks of 128

    nc = bacc.Bacc("TRN2", target_bir_lowering=False, debug=False)

    xT_d = nc.dram_tensor("xT", [E, NPADC], F16, kind="ExternalInput").ap()
    OT_d = nc.dram_tensor("OT", [NT // DB, T, DB, SPB], F16, kind="ExternalInput").ap()
    OG_d = nc.dram_tensor("OG", [NT // DB, SPB, DB, T], F16, kind="ExternalInput").ap()
    ysT_d = nc.dram_tensor("ysT", [YD, B_LOC], F16, kind="ExternalInput").ap()
    CW_d = nc.dram_tensor("CW", [YD, E], F16, kind="ExternalInput").ap()
    W1X_d, W1G_d, W2_d, GW1_d, GW2_d = {}, {}, {}, {}, {}
    for l in range(3):
        W1X_d[l] = nc.dram_tensor(f"W1X{l}", [E, H], F16, kind="ExternalInput").ap()
        W2_d[l] = nc.dram_tensor(f"W2_{l}", [H, E], F16, kind="ExternalInput").ap()
        GW1_d[l] = nc.dram_tensor(f"GW1_{l}", [E, H], F16, kind="ExternalInput").ap()
        GW2_d[l] = nc.dram_tensor(f"GW2_{l}", [H, E], F16, kind="ExternalInput").ap()
        if l > 0:
            W1G_d[l] = nc.dram_tensor(f"W1G{l}", [E, H], F16, kind="ExternalInput").ap()
    outT_d = nc.dram_tensor("outT", [E, B_LOC], F32, kind="ExternalOutput").ap()

    with tile.TileContext(nc) as tc:
        with tc.tile_pool(name="const", bufs=1) as cpool, \
             tc.tile_pool(name="xin", bufs=3) as xpool, \
             tc.tile_pool(name="otin", bufs=3) as otpool, \
             tc.tile_pool(name="ogin", bufs=3) as ogpool, \
             tc.tile_pool(name="rstat", bufs=4) as spool, \
             tc.tile_pool(name="relu", bufs=6) as rpool, \
             tc.tile_pool(name="af", bufs=SB + 6) as afpool, \
             tc.tile_pool(name="scr", bufs=2) as scrpool, \
             tc.tile_pool(name="segsb", bufs=3) as segsb, \
             tc.tile_pool(name="a1ps", bufs=4, space="PSUM") as a1pool, \
             tc.tile_pool(name="zps", bufs=2, space="PSUM") as zpool, \
             tc.tile_pool(name="segps", bufs=2, space="PSUM") as segps:

            # ---- resident constants ----
            def load_const(name, dram_ap, shape, rearr=None):
                tl = cpool.tile(shape, F16, tag=name)
                src = dram_ap if rearr is None else dram_ap.rearrange(rearr, c=HC)
                nc.sync.dma_start(tl[:], src)
                return tl

            w1x = {l: load_const(f"w1x{l}", W1X_d[l], [E, H]) for l in range(3)}
            w1g = {l: load_const(f"w1g{l}", W1G_d[l], [E, H]) for l in (1, 2)}
            gw1 = {l: load_const(f"gw1{l}", GW1_d[l], [E, H]) for l in range(3)}
            # w2 / gw2 as [128, HC, E] chunked stationary operands
            w2 = {l: load_const(f"w2{l}", W2_d[l], [128, HC, E], "(c p) e -> p c e")
                  for l in range(3)}
            gw2 = {l: load_const(f"gw2{l}", GW2_d[l], [128, HC, E], "(c p) e -> p c e")
                   for l in range(3)}
            cw = load_const("cw", CW_d, [YD, E])
            ys = load_const("ys", ysT_d, [YD, B_LOC])
            ident = cpool.tile([128, 128], F16, tag="ident")
            make_identity(nc, ident[:])
            eps_c = cpool.tile([128, 1], F32, tag="eps_c")
            nc.gpsimd.memset(eps_c[:], LN_EPS)
            xgw_store = cpool.tile([128, NBLK, H], F16, tag="xgw")

            tcount = 0
            for l in range(3):
                for blk in range(NBLK):
                    z = zpool.tile([SPB, H], F32, tag="z")
                    for sbi in range(MAXBLK // SB):
                        ss = spool.tile([T, SB], F32, tag="ss")
                        a1fs, ots = [], []
                        for db in range(SB // DB):
                            tbi = (blk * MAXBLK + sbi * SB + db * DB) // DB
                            node0 = tbi * DB * T
                            xt = xpool.tile([E, DB * T], F16, tag="xt")
                            nc.sync.dma_start(xt[:], xT_d[:, node0:node0 + DB * T])
                            ot = otpool.tile([T, DB, SPB], F16, tag="ot")
                            nc.sync.dma_start(ot[:], OT_d[tbi])
                            if l > 0:
                                og = ogpool.tile([SPB, DB, T], F16, tag="og")
                                nc.sync.dma_start(og[:], OG_d[tbi])
                            for j in range(DB):
                                g = db * DB + j
                                a1 = a1pool.tile([T, H], F32, tag="a1")
                                nc.tensor.matmul(a1[:], xt[:, j * T:(j + 1) * T],
                                                 w1x[l][:], start=True, stop=(l == 0))
                                if l > 0:
                                    nc.tensor.matmul(a1[:], og[:, j, :],
                                                     xgw_store[:, blk, :],
                                                     start=False, stop=True)
                                a1f = afpool.tile([T, H], F16, tag="a1f")
                                if tcount % 4 == 3:
                                    nc.vector.tensor_copy(a1f[:], a1[:])
                                else:
                                    nc.scalar.copy(a1f[:], a1[:])
                                tcount += 1
                                sq = scrpool.tile([T, H], F16, tag="sq")
                                nc.vector.scalar_tensor_tensor(
                                    out=sq[:], in0=a1f[:], scalar=1.0, in1=a1f[:],
                                    op0=ALU.mult, op1=ALU.mult,
                                    accum_out=ss[:, g:g + 1])
                                a1fs.append(a1f)
                                ots.append((ot, j))
                        sd = spool.tile([T, SB], F32, tag="sd")
                        nc.scalar.activation(sd[:], ss[:], ACTF.Sqrt,
                                             bias=eps_c[:], scale=1.0 / H)
                        r4 = spool.tile([T, SB], F32, tag="r4")
                        nc.vector.reciprocal(r4[:], sd[:])
                        for g in range(SB):
                            R = rpool.tile([T, H], F16, tag="R")
                            nc.gpsimd.tensor_scalar(
                                R[:], a1fs[g][:], r4[:, g:g + 1], 0.0,
                                ALU.mult, ALU.max)
                            oth, oj = ots[g]
                            nc.tensor.matmul(z[:], oth[:, oj, :], R[:],
                                             start=(sbi == 0 and g == 0),
                                             stop=(sbi == MAXBLK // SB - 1 and g == SB - 1))
                    # ---- segment phase for this block ----
                    z_sb = segsb.tile([SPB, H], F16, tag="z_sb")
                    nc.scalar.copy(z_sb[:], z[:])
                    zT = segps.tile([128, HC, SPB], F16, tag="segps")
                    for c in range(HC):
                        nc.tensor.transpose(zT[:, c, :], z_sb[:, c * 128:(c + 1) * 128], ident[:])
                    zT_sb = segsb.tile([128, HC, SPB], F16, tag="zT_sb")
                    nc.vector.tensor_copy(zT_sb[:], zT[:])
                    sT = segps.tile([E, SPB], F32, tag="segps")
                    for c in range(HC):
                        nc.tensor.matmul(sT[:], w2[l][:, c, :], zT_sb[:, c, :],
                                         start=(c == 0), stop=(c == HC - 1))
                    sT_sb = segsb.tile([E, SPB], F16, tag="sT_sb")
                    nc.scalar.copy(sT_sb[:], sT[:])
                    ug = segps.tile([SPB, H], F32, tag="segps")
                    nc.tensor.matmul(ug[:], sT_sb[:], gw1[l][:], start=True, stop=True)
                    ssg = spool.tile([SPB, 1], F32, tag="ssg")
                    sqg = scrpool.tile([SPB, H], F16, tag="sq")
                    nc.scalar.activation(sqg[:], ug[:], ACTF.Square,
                                         accum_out=ssg[:])
                    sdg = spool.tile([SPB, 1], F32, tag="sdg")
                    nc.scalar.activation(sdg[:], ssg[:], ACTF.Sqrt,
                                         bias=eps_c[:], scale=1.0 / H)
                    rg = spool.tile([SPB, 1], F32, tag="rg")
                    nc.vector.reciprocal(rg[:], sdg[:])
                    Rg = segsb.tile([SPB, H], F16, tag="Rg")
                    nc.vector.tensor_scalar(
                        Rg[:], ug[:], rg[:], 0.0, ALU.mult, ALU.max)
                    RgT = segps.tile([128, HC, SPB], F16, tag="segps")
                    for c in range(HC):
                        nc.tensor.transpose(RgT[:, c, :], Rg[:, c * 128:(c + 1) * 128], ident[:])
                    RgT_sb = segsb.tile([128, HC, SPB], F16, tag="RgT_sb")
                    nc.vector.tensor_copy(RgT_sb[:], RgT[:])
                    xgT = segps.tile([E, SPB], F32, tag="segps")
                    for c in range(HC):
                        nc.tensor.matmul(xgT[:], gw2[l][:, c, :], RgT_sb[:, c, :],
                                         start=(c == 0),
                                         stop=(c == HC - 1 and l > 0))
                    if l == 0:
                        nc.tensor.matmul(xgT[:], cw[:], ys[:, blk * SPB:(blk + 1) * SPB],
                                         start=False, stop=True)
                    if l < 2:
                        xgT_sb = segsb.tile([E, SPB], F16, tag="xgT_sb")
                        nc.vector.tensor_copy(xgT_sb[:], xgT[:])
                        xgw = segps.tile([SPB, H], F32, tag="segps")
                        nc.tensor.matmul(xgw[:], xgT_sb[:], w1g[l + 1][:],
                                         start=True, stop=True)
                        nc.scalar.copy(xgw_store[:, blk, :], xgw[:])
                    else:
                        o_sb = segsb.tile([E, SPB], F32, tag="o_sb")
                        nc.vector.tensor_copy(o_sb[:], xgT[:])
                        nc.sync.dma_start(outT_d[:, blk * SPB:(blk + 1) * SPB], o_sb[:])

    nc.compile()
    return nc


def _run(inputs, trace=False):
    geom, shared, percore = _prep_host(inputs)
    nc = _build_program(geom)
    in_maps = []
    for c in range(NCORES):
        m = dict(shared)
        m.update(percore[c])
        in_maps.append(m)
    res = run_bass_kernel_spmd(nc, in_maps, list(range(NCORES)), trace=trace)
    B, E, B_LOC = geom["B"], geom["E"], geom["B_LOC"]
    out = np.empty((B, E), np.float32)
    for c in range(NCORES):
        out[c * B_LOC:(c + 1) * B_LOC] = res.results[c]["outT"].T
    return out, res


def kernel(**inputs):
    out, _ = _run(inputs)
    return out



# revision 7
# speedup vs baseline: 4.9971x; 4.9971x over previous
"""DeeperSet aggregation kernel for 8 Trainium2 NeuronCores.

Strategy: data-parallel over contiguous graph-id ranges (2048 graphs/core).
Segment boundaries are host-known (batch is an input), so segment-sum and
the xg[batch] gather are expressed as matmuls against host-built one-hot
tiles.  LayerNorm (gamma=1, beta=0, biases=0 in this model) reduces to a
per-node positive scale r = 1/sqrt(mean(u^2)+eps) with mean-centering folded
into the weights on the host; r commutes through ReLU and the second linear,
so it is applied inside the ReLU activation (ACT per-partition scale).
"""

import sys

sys.path.insert(0, "/opt/trn_rl_repo")

import numpy as np

import concourse.bass as bass
import concourse.tile as tile
from concourse import bacc, mybir
from concourse.bass_utils import run_bass_kernel_spmd
from concourse.masks import make_identity

F32 = mybir.dt.float32
F16 = mybir.dt.float16
ALU = mybir.AluOpType
ACTF = mybir.ActivationFunctionType

LN_EPS = 1e-5
NCORES = 8
SPB = 128          # segments (graphs) per block
T = 128            # nodes per tile
SB = 16            # tiles per stats batch
DB = 8             # tiles per DMA batch


def _center(w, g):
    return ((w - w.mean(axis=1, keepdims=True)) * g[None, :]).astype(np.float32)


def _prep_host(inputs):
    x = np.asarray(inputs["x"], np.float32)
    y = np.asarray(inputs["y"], np.float32)
    batch = np.asarray(inputs["batch"], np.int64)
    N, E = x.shape
    B, YD = y.shape
    H = inputs["l0_lw1"].shape[1]

    for k in ("l0_lb1", "l0_lbt", "l0_lb2", "l0_gb1", "l0_gbt", "l0_gb2",
              "lr_lb1", "lr_lbt", "lr_lb2", "lr_gb1", "lr_gbt", "lr_gb2", "cb"):
        assert np.abs(np.asarray(inputs[k])).max() < 1e-12, f"{k} must be zero"
    for k in ("l0_lg", "l0_gg", "lr_lg", "lr_gg"):
        assert np.abs(np.asarray(inputs[k]) - 1.0).max() < 1e-12, f"{k} must be one"

    B_LOC = B // NCORES
    NBLK = B_LOC // SPB
    nblocks = B // SPB
    edges = np.searchsorted(batch, np.arange(0, B + 1, SPB)).astype(np.int64)
    cnts = np.diff(edges)
    maxblk = int(np.ceil(cnts.max() / T)) if N > 0 else 1
    MAXBLK = max(SB, ((maxblk + SB - 1) // SB) * SB)
    NT = NBLK * MAXBLK          # tiles per core
    NPADC = NT * T              # padded nodes per core

    xT = [np.zeros((E, NPADC), np.float16) for _ in range(NCORES)]
    OT = [np.zeros((NT // DB, T, DB, SPB), np.float16) for _ in range(NCORES)]
    OG = [np.zeros((NT // DB, SPB, DB, T), np.float16) for _ in range(NCORES)]
    ysT = [None] * NCORES
    for c in range(NCORES):
        for k in range(NBLK):
            j = c * NBLK + k
            n0, n1 = int(edges[j]), int(edges[j + 1])
            cnt = n1 - n0
            if cnt == 0:
                continue
            base = k * MAXBLK * T
            xT[c][:, base:base + cnt] = x[n0:n1].T.astype(np.float16)
            a = base + np.arange(cnt)
            t = a // T
            p = a % T
            g = (batch[n0:n1] - j * SPB).astype(np.int64)
            OT[c][t // DB, p, t % DB, g] = 1.0
            OG[c][t // DB, g, t % DB, p] = 1.0
        ysT[c] = np.ascontiguousarray(y[c * B_LOC:(c + 1) * B_LOC].T).astype(np.float16)

    f16 = lambda w: np.ascontiguousarray(w).astype(np.float16)
    l0_w1f = _center(np.asarray(inputs["l0_lw1"], np.float32), np.asarray(inputs["l0_lg"], np.float32))
    W1X, W1G = [f16(l0_w1f)], [None]
    W2 = [f16(np.asarray(inputs["l0_lw2"], np.float32))]
    GW1 = [f16(_center(np.asarray(inputs["l0_gw1"], np.float32), np.asarray(inputs["l0_gg"], np.float32)))]
    GW2 = [f16(np.asarray(inputs["l0_gw2"], np.float32))]
    for i in range(2):
        w1f = _center(np.asarray(inputs["lr_lw1"][i], np.float32), np.asarray(inputs["lr_lg"][i], np.float32))
        W1X.append(f16(w1f[:E]))
        W1G.append(f16(w1f[E:]))
        W2.append(f16(np.asarray(inputs["lr_lw2"][i], np.float32)))
        GW1.append(f16(_center(np.asarray(inputs["lr_gw1"][i], np.float32), np.asarray(inputs["lr_gg"][i], np.float32))))
        GW2.append(f16(np.asarray(inputs["lr_gw2"][i], np.float32)))
    CW = f16(np.asarray(inputs["cw"], np.float32))

    geom = dict(N=N, E=E, B=B, YD=YD, H=H, B_LOC=B_LOC, NBLK=NBLK,
                MAXBLK=MAXBLK, NT=NT, NPADC=NPADC)
    shared = dict(CW=CW)
    for l in range(3):
        shared[f"W1X{l}"] = W1X[l]
        shared[f"W2_{l}"] = W2[l]
        shared[f"GW1_{l}"] = GW1[l]
        shared[f"GW2_{l}"] = GW2[l]
        if l > 0:
            shared[f"W1G{l}"] = W1G[l]
    percore = [dict(xT=xT[c], OT=OT[c], OG=OG[c], ysT=ysT[c]) for c in range(NCORES)]
    return geom, shared, percore


def _build_program(geom):
    E, H, YD = geom["E"], geom["H"], geom["YD"]
    B_LOC, NBLK, MAXBLK, NT, NPADC = (geom["B_LOC"], geom["NBLK"],
                                      geom["MAXBLK"], geom["NT"], geom["NPADC"])
    HC = H // 128  # H chunks of 128

    nc = bacc.Bacc("TRN2", target_bir_lowering=False, debug=False)

    xT_d = nc.dram_tensor("xT", [E, NPADC], F16, kind="ExternalInput").ap()
    OT_d = nc.dram_tensor("OT", [NT // DB, T, DB, SPB], F16, kind="ExternalInput").ap()
    OG_d = nc.dram_tensor("OG", [NT // DB, SPB, DB, T], F16, kind="ExternalInput").ap()
    ysT_d = nc.dram_tensor("ysT", [YD, B_LOC], F16, kind="ExternalInput").ap()
    CW_d = nc.dram_tensor("CW", [YD, E], F16, kind="ExternalInput").ap()
    W1X_d, W1G_d, W2_d, GW1_d, GW2_d = {}, {}, {}, {}, {}
    for l in range(3):
        W1X_d[l] = nc.dram_tensor(f"W1X{l}", [E, H], F16, kind="ExternalInput").ap()
        W2_d[l] = nc.dram_tensor(f"W2_{l}", [H, E], F16, kind="ExternalInput").ap()
        GW1_d[l] = nc.dram_tensor(f"GW1_{l}", [E, H], F16, kind="ExternalInput").ap()
        GW2_d[l] = nc.dram_tensor(f"GW2_{l}", [H, E], F16, kind="ExternalInput").ap()
        if l > 0:
            W1G_d[l] = nc.dram_tensor(f"W1G{l}", [E, H], F16, kind="ExternalInput").ap()
    outT_d = nc.dram_tensor("outT", [E, B_LOC], F32, kind="ExternalOutput").ap()

    with tile.TileContext(nc) as tc:
        with tc.tile_pool(name="const", bufs=1) as cpool, \
             tc.tile_pool(name="xin", bufs=3) as xpool, \
             tc.tile_pool(name="otin", bufs=3) as otpool, \
             tc.tile_pool(name="ogin", bufs=3) as ogpool, \
             tc.tile_pool(name="rstat", bufs=4) as spool, \
             tc.tile_pool(name="relu", bufs=6) as rpool, \
             tc.tile_pool(name="af", bufs=SB + 6) as afpool, \
             tc.tile_pool(name="scr", bufs=2) as scrpool, \
             tc.tile_pool(name="segsb", bufs=3) as segsb, \
             tc.tile_pool(name="a1ps", bufs=4, space="PSUM") as a1pool, \
             tc.tile_pool(name="zps", bufs=2, space="PSUM") as zpool, \
             tc.tile_pool(name="segps", bufs=2, space="PSUM") as segps:

            # ---- resident constants ----
            def load_const(name, dram_ap, shape, rearr=None):
                tl = cpool.tile(shape, F16, tag=name)
                src = dram_ap if rearr is None else dram_ap.rearrange(rearr, c=HC)
                nc.sync.dma_start(tl[:], src)
                return tl

            w1x = {l: load_const(f"w1x{l}", W1X_d[l], [E, H]) for l in range(3)}
            w1g = {l: load_const(f"w1g{l}", W1G_d[l], [E, H]) for l in (1, 2)}
            gw1 = {l: load_const(f"gw1{l}", GW1_d[l], [E, H]) for l in range(3)}
            # w2 / gw2 as [128, HC, E] chunked stationary operands
            w2 = {l: load_const(f"w2{l}", W2_d[l], [128, HC, E], "(c p) e -> p c e")
                  for l in range(3)}
            gw2 = {l: load_const(f"gw2{l}", GW2_d[l], [128, HC, E], "(c p) e -> p c e")
                   for l in range(3)}
            cw = load_const("cw", CW_d, [YD, E])
            ys = load_const("ys", ysT_d, [YD, B_LOC])
            ident = cpool.tile([128, 128], F16, tag="ident")
            make_identity(nc, ident[:])
            eps_c = cpool.tile([128, 1], F32, tag="eps_c")
            nc.gpsimd.memset(eps_c[:], LN_EPS)
            xgw_store = cpool.tile([128, NBLK, H], F16, tag="xgw")

            for l in range(3):
                for blk in range(NBLK):
                    z = zpool.tile([SPB, H], F32, tag="z")
                    for sbi in range(MAXBLK // SB):
                        ss = spool.tile([T, SB], F32, tag="ss")
                        a1fs, ots = [], []
                        for db in range(SB // DB):
                            tbi = (blk * MAXBLK + sbi * SB + db * DB) // DB
                            node0 = tbi * DB * T
                            xt = xpool.tile([E, DB * T], F16, tag="xt")
                            nc.sync.dma_start(xt[:], xT_d[:, node0:node0 + DB * T])
                            ot = otpool.tile([T, DB, SPB], F16, tag="ot")
                            nc.sync.dma_start(ot[:], OT_d[tbi])
                            if l > 0:
                                og = ogpool.tile([SPB, DB, T], F16, tag="og")
                                nc.sync.dma_start(og[:], OG_d[tbi])
                            for j in range(DB):
                                g = db * DB + j
                                a1 = a1pool.tile([T, H], F32, tag="a1")
                                nc.tensor.matmul(a1[:], xt[:, j * T:(j + 1) * T],
                                                 w1x[l][:], start=True, stop=(l == 0))
                                if l > 0:
                                    nc.tensor.matmul(a1[:], og[:, j, :],
                                                     xgw_store[:, blk, :],
                                                     start=False, stop=True)
                                sq = scrpool.tile([T, H], F16, tag="sq")
                                nc.scalar.activation(sq[:], a1[:], ACTF.Square,
                                                     accum_out=ss[:, g:g + 1])
                                a1f = afpool.tile([T, H], F16, tag="a1f")
                                nc.vector.tensor_scalar(
                                    a1f[:], a1[:], 1.0, 0.0, ALU.mult, ALU.max)
                                a1fs.append(a1f)
                                ots.append((ot, j))
                        sd = spool.tile([T, SB], F32, tag="sd")
                        nc.scalar.activation(sd[:], ss[:], ACTF.Sqrt,
                                             bias=eps_c[:], scale=1.0 / H)
                        r4 = spool.tile([T, SB], F32, tag="r4")
                        nc.vector.reciprocal(r4[:], sd[:])
                        for g in range(SB):
                            oth, oj = ots[g]
                            otr = rpool.tile([T, SPB], F16, tag="R")
                            nc.vector.tensor_scalar(
                                otr[:], oth[:, oj, :], r4[:, g:g + 1], 0.0,
                                ALU.mult, ALU.max)
                            nc.tensor.matmul(z[:], otr[:], a1fs[g][:],
                                             start=(sbi == 0 and g == 0),
                                             stop=(sbi == MAXBLK // SB - 1 and g == SB - 1))
                    # ---- segment phase for this block ----
                    z_sb = segsb.tile([SPB, H], F16, tag="z_sb")
                    nc.scalar.copy(z_sb[:], z[:])
                    zT = segps.tile([128, HC, SPB], F16, tag="segps")
                    for c in range(HC):
                        nc.tensor.transpose(zT[:, c, :], z_sb[:, c * 128:(c + 1) * 128], ident[:])
                    zT_sb = segsb.tile([128, HC, SPB], F16, tag="zT_sb")
                    nc.vector.tensor_copy(zT_sb[:], zT[:])
                    sT = segps.tile([E, SPB], F32, tag="segps")
                    for c in range(HC):
                        nc.tensor.matmul(sT[:], w2[l][:, c, :], zT_sb[:, c, :],
                                         start=(c == 0), stop=(c == HC - 1))
                    sT_sb = segsb.tile([E, SPB], F16, tag="sT_sb")
                    nc.scalar.copy(sT_sb[:], sT[:])
                    ug = segps.tile([SPB, H], F32, tag="segps")
                    nc.tensor.matmul(ug[:], sT_sb[:], gw1[l][:], start=True, stop=True)
                    ssg = spool.tile([SPB, 1], F32, tag="ssg")
                    sqg = scrpool.tile([SPB, H], F16, tag="sq")
                    nc.scalar.activation(sqg[:], ug[:], ACTF.Square,
                                         accum_out=ssg[:])
                    sdg = spool.tile([SPB, 1], F32, tag="sdg")
                    nc.scalar.activation(sdg[:], ssg[:], ACTF.Sqrt,
                                         bias=eps_c[:], scale=1.0 / H)
                    rg = spool.tile([SPB, 1], F32, tag="rg")
                    nc.vector.reciprocal(rg[:], sdg[:])
                    Rg = segsb.tile([SPB, H], F16, tag="Rg")
                    nc.vector.tensor_scalar(
                        Rg[:], ug[:], rg[:], 0.0, ALU.mult, ALU.max)
                    RgT = segps.tile([128, HC, SPB], F16, tag="segps")
                    for c in range(HC):
                        nc.tensor.transpose(RgT[:, c, :], Rg[:, c * 128:(c + 1) * 128], ident[:])
                    RgT_sb = segsb.tile([128, HC, SPB], F16, tag="RgT_sb")
                    nc.vector.tensor_copy(RgT_sb[:], RgT[:])
                    xgT = segps.tile([E, SPB], F32, tag="segps")
                    for c in range(HC):
                        nc.tensor.matmul(xgT[:], gw2[l][:, c, :], RgT_sb[:, c, :],
                                         start=(c == 0),
                                         stop=(c == HC - 1 and l > 0))
                    if l == 0:
                        nc.tensor.matmul(xgT[:], cw[:], ys[:, blk * SPB:(blk + 1) * SPB],
                                         start=False, stop=True)
                    if l < 2:
                        xgT_sb = segsb.tile([E, SPB], F16, tag="xgT_sb")
                        nc.vector.tensor_copy(xgT_sb[:], xgT[:])
                        xgw = segps.tile([SPB, H], F32, tag="segps")
                        nc.tensor.matmul(xgw[:], xgT_sb[:], w1g[l + 1][:],
                                         start=True, stop=True)
                        nc.scalar.copy(xgw_store[:, blk, :], xgw[:])
                    else:
                        o_sb = segsb.tile([E, SPB], F32, tag="o_sb")
                        nc.vector.tensor_copy(o_sb[:], xgT[:])
                        nc.sync.dma_start(outT_d[:, blk * SPB:(blk + 1) * SPB], o_sb[:])

    nc.compile()
    return nc


def _run(inputs, trace=False):
    geom, shared, percore = _prep_host(inputs)
    nc = _build_program(geom)
    in_maps = []
    for c in range(NCORES):
        m = dict(shared)
        m.update(percore[c])
        in_maps.append(m)
    res = run_bass_kernel_spmd(nc, in_maps, list(range(NCORES)), trace=trace)
    B, E, B_LOC = geom["B"], geom["E"], geom["B_LOC"]
    out = np.empty((B, E), np.float32)
    for c in range(NCORES):
        out[c * B_LOC:(c + 1) * B_LOC] = res.results[c]["outT"].T
    return out, res


def kernel(**inputs):
    out, _ = _run(inputs)
    return out



# revision 9
# speedup vs baseline: 6.4004x; 1.2808x over previous
"""DeeperSet aggregation kernel for 8 Trainium2 NeuronCores.

Strategy: data-parallel over contiguous graph-id ranges (2048 graphs/core).
Segment boundaries are host-known (batch is an input), so segment-sum and
the xg[batch] gather are expressed as matmuls against host-built one-hot
tiles.  LayerNorm (gamma=1, beta=0, biases=0 in this model) reduces to a
per-node positive scale r = 1/sqrt(mean(u^2)+eps) with mean-centering folded
into the weights on the host.  r commutes through ReLU and the segment-sum,
so it is applied to the (half-width) one-hot rows instead of the
activations.  Elementwise work is batched over GRP-tile super-groups to
amortize per-instruction overheads, and emission is software-pipelined
(one stats-batch and one segment-phase of lookahead) so the PE never
stalls and ramps to full clock.
"""

import sys

sys.path.insert(0, "/opt/trn_rl_repo")

import numpy as np

import concourse.bass as bass
import concourse.tile as tile
from concourse import bacc, mybir
from concourse.bass_utils import run_bass_kernel_spmd
from concourse.masks import make_identity

F32 = mybir.dt.float32
F16 = mybir.dt.float16
ALU = mybir.AluOpType
ACTF = mybir.ActivationFunctionType
AXL = mybir.AxisListType

LN_EPS = 1e-5
NCORES = 8
SPB = 128          # segments (graphs) per block
T = 128            # nodes per tile
GRP = 4            # tiles per elementwise super-group
SB = 8             # tiles per stats batch (= 2 groups)


def _center(w, g):
    return ((w - w.mean(axis=1, keepdims=True)) * g[None, :]).astype(np.float32)


def _prep_host(inputs):
    x = np.asarray(inputs["x"], np.float32)
    y = np.asarray(inputs["y"], np.float32)
    batch = np.asarray(inputs["batch"], np.int64)
    N, E = x.shape
    B, YD = y.shape
    H = inputs["l0_lw1"].shape[1]

    for k in ("l0_lb1", "l0_lbt", "l0_lb2", "l0_gb1", "l0_gbt", "l0_gb2",
              "lr_lb1", "lr_lbt", "lr_lb2", "lr_gb1", "lr_gbt", "lr_gb2", "cb"):
        assert np.abs(np.asarray(inputs[k])).max() < 1e-12, f"{k} must be zero"
    for k in ("l0_lg", "l0_gg", "lr_lg", "lr_gg"):
        assert np.abs(np.asarray(inputs[k]) - 1.0).max() < 1e-12, f"{k} must be one"

    B_LOC = B // NCORES
    NBLK = B_LOC // SPB
    edges = np.searchsorted(batch, np.arange(0, B + 1, SPB)).astype(np.int64)
    cnts = np.diff(edges)
    maxblk = int(np.ceil(cnts.max() / T)) if N > 0 else 1
    MAXBLK = max(SB, ((maxblk + SB - 1) // SB) * SB)
    NT = NBLK * MAXBLK          # tiles per core
    NPADC = NT * T              # padded nodes per core

    xT = [np.zeros((E, NPADC), np.float16) for _ in range(NCORES)]
    OT = [np.zeros((NBLK, T, MAXBLK, SPB), np.float16) for _ in range(NCORES)]
    OG = [np.zeros((NBLK, SPB, MAXBLK, T), np.float16) for _ in range(NCORES)]
    ysT = [None] * NCORES
    for c in range(NCORES):
        for k in range(NBLK):
            j = c * NBLK + k
            n0, n1 = int(edges[j]), int(edges[j + 1])
            cnt = n1 - n0
            if cnt == 0:
                continue
            base = k * MAXBLK * T
            xT[c][:, base:base + cnt] = x[n0:n1].T.astype(np.float16)
            a = np.arange(cnt)
            t = a // T
            p = a % T
            g = (batch[n0:n1] - j * SPB).astype(np.int64)
            OT[c][k, p, t, g] = 1.0
            OG[c][k, g, t, p] = 1.0
        ysT[c] = np.ascontiguousarray(y[c * B_LOC:(c + 1) * B_LOC].T).astype(np.float16)

    f16 = lambda w: np.ascontiguousarray(w).astype(np.float16)
    l0_w1f = _center(np.asarray(inputs["l0_lw1"], np.float32), np.asarray(inputs["l0_lg"], np.float32))
    W1X, W1G = [f16(l0_w1f)], [None]
    W2 = [f16(np.asarray(inputs["l0_lw2"], np.float32))]
    GW1 = [f16(_center(np.asarray(inputs["l0_gw1"], np.float32), np.asarray(inputs["l0_gg"], np.float32)))]
    GW2 = [f16(np.asarray(inputs["l0_gw2"], np.float32))]
    for i in range(2):
        w1f = _center(np.asarray(inputs["lr_lw1"][i], np.float32), np.asarray(inputs["lr_lg"][i], np.float32))
        W1X.append(f16(w1f[:E]))
        W1G.append(f16(w1f[E:]))
        W2.append(f16(np.asarray(inputs["lr_lw2"][i], np.float32)))
        GW1.append(f16(_center(np.asarray(inputs["lr_gw1"][i], np.float32), np.asarray(inputs["lr_gg"][i], np.float32))))
        GW2.append(f16(np.asarray(inputs["lr_gw2"][i], np.float32)))
    CW = f16(np.asarray(inputs["cw"], np.float32))

    geom = dict(N=N, E=E, B=B, YD=YD, H=H, B_LOC=B_LOC, NBLK=NBLK,
                MAXBLK=MAXBLK, NT=NT, NPADC=NPADC)
    shared = dict(CW=CW)
    for l in range(3):
        shared[f"W1X{l}"] = W1X[l]
        shared[f"W2_{l}"] = W2[l]
        shared[f"GW1_{l}"] = GW1[l]
        shared[f"GW2_{l}"] = GW2[l]
        if l > 0:
            shared[f"W1G{l}"] = W1G[l]
    percore = [dict(xT=xT[c], OT=OT[c], OG=OG[c], ysT=ysT[c]) for c in range(NCORES)]
    return geom, shared, percore


def _build_program(geom):
    E, H, YD = geom["E"], geom["H"], geom["YD"]
    B_LOC, NBLK, MAXBLK, NT, NPADC = (geom["B_LOC"], geom["NBLK"],
                                      geom["MAXBLK"], geom["NT"], geom["NPADC"])
    HC = H // 128  # H chunks of 128
    NSB = MAXBLK // SB

    nc = bacc.Bacc("TRN2", target_bir_lowering=False, debug=False)

    xT_d = nc.dram_tensor("xT", [E, NPADC], F16, kind="ExternalInput").ap()
    OT_d = nc.dram_tensor("OT", [NBLK, T, MAXBLK, SPB], F16, kind="ExternalInput").ap()
    OG_d = nc.dram_tensor("OG", [NBLK, SPB, MAXBLK, T], F16, kind="ExternalInput").ap()
    ysT_d = nc.dram_tensor("ysT", [YD, B_LOC], F16, kind="ExternalInput").ap()
    CW_d = nc.dram_tensor("CW", [YD, E], F16, kind="ExternalInput").ap()
    W1X_d, W1G_d, W2_d, GW1_d, GW2_d = {}, {}, {}, {}, {}
    for l in range(3):
        W1X_d[l] = nc.dram_tensor(f"W1X{l}", [E, H], F16, kind="ExternalInput").ap()
        W2_d[l] = nc.dram_tensor(f"W2_{l}", [H, E], F16, kind="ExternalInput").ap()
        GW1_d[l] = nc.dram_tensor(f"GW1_{l}", [E, H], F16, kind="ExternalInput").ap()
        GW2_d[l] = nc.dram_tensor(f"GW2_{l}", [H, E], F16, kind="ExternalInput").ap()
        if l > 0:
            W1G_d[l] = nc.dram_tensor(f"W1G{l}", [E, H], F16, kind="ExternalInput").ap()
    outT_d = nc.dram_tensor("outT", [E, B_LOC], F32, kind="ExternalOutput").ap()

    with tile.TileContext(nc) as tc:
        with tc.tile_pool(name="const", bufs=1) as cpool, \
             tc.tile_pool(name="xin", bufs=2) as xpool, \
             tc.tile_pool(name="otin", bufs=2) as otpool, \
             tc.tile_pool(name="ogin", bufs=2) as ogpool, \
             tc.tile_pool(name="rstat", bufs=8) as spool, \
             tc.tile_pool(name="otr", bufs=4) as rpool, \
             tc.tile_pool(name="sqs", bufs=3) as sqpool, \
             tc.tile_pool(name="af", bufs=6) as afpool, \
             tc.tile_pool(name="segsb", bufs=3) as segsb, \
             tc.tile_pool(name="a1ps", bufs=2, space="PSUM") as a1pool, \
             tc.tile_pool(name="zps", bufs=2, space="PSUM") as zpool, \
             tc.tile_pool(name="segps", bufs=2, space="PSUM") as segps:

            # ---- resident constants ----
            def load_const(name, dram_ap, shape, rearr=None):
                tl = cpool.tile(shape, F16, tag=name)
                src = dram_ap if rearr is None else dram_ap.rearrange(rearr, c=HC)
                nc.sync.dma_start(tl[:], src)
                return tl

            w1x = {l: load_const(f"w1x{l}", W1X_d[l], [E, H]) for l in range(3)}
            w1g = {l: load_const(f"w1g{l}", W1G_d[l], [E, H]) for l in (1, 2)}
            gw1 = {l: load_const(f"gw1{l}", GW1_d[l], [E, H]) for l in range(3)}
            # w2 / gw2 as [128, HC, E] chunked stationary operands
            w2 = {l: load_const(f"w2{l}", W2_d[l], [128, HC, E], "(c p) e -> p c e")
                  for l in range(3)}
            gw2 = {l: load_const(f"gw2{l}", GW2_d[l], [128, HC, E], "(c p) e -> p c e")
                   for l in range(3)}
            cw = load_const("cw", CW_d, [YD, E])
            ys = load_const("ys", ysT_d, [YD, B_LOC])
            ident = cpool.tile([128, 128], F16, tag="ident")
            make_identity(nc, ident[:])
            eps_c = cpool.tile([128, 1], F32, tag="eps_c")
            nc.gpsimd.memset(eps_c[:], LN_EPS)
            xgw_store = cpool.tile([128, NBLK, H], F16, tag="xgw")

            gcount = [0]

            def back_batch(sbi, ss_b, groups, ot, z):
                """Stats + one-hot scaling + segment-sum matmuls for a
                completed stats batch (emitted one batch late)."""
                sd = spool.tile([T, SB], F32, tag="sd")
                nc.scalar.activation(sd[:], ss_b[:], ACTF.Sqrt,
                                     bias=eps_c[:], scale=1.0 / H)
                r4 = spool.tile([T, SB], F32, tag="r4")
                nc.vector.reciprocal(r4[:], sd[:])
                for gi, af in enumerate(groups):
                    for j in range(GRP):
                        g = gi * GRP + j
                        ti = sbi * SB + g
                        otr = rpool.tile([T, SPB], F16, tag="otr")
                        nc.vector.tensor_scalar(
                            otr[:], ot[:, ti, :], r4[:, g:g + 1], 0.0,
                            ALU.mult, ALU.max)
                        nc.tensor.matmul(z[:], otr[:], af[:, j, :],
                                         start=(sbi == 0 and g == 0),
                                         stop=(sbi == NSB - 1 and g == SB - 1))

            def seg_phase(l, blk, z):
                """Per-block segment pipeline: z -> s -> global MLP -> xg."""
                z_sb = segsb.tile([SPB, H], F16, tag="z_sb")
                nc.scalar.copy(z_sb[:], z[:])
                zT = segps.tile([128, HC, SPB], F16, tag="segps")
                for c in range(HC):
                    nc.tensor.transpose(zT[:, c, :], z_sb[:, c * 128:(c + 1) * 128], ident[:])
                zT_sb = segsb.tile([128, HC, SPB], F16, tag="zT_sb")
                nc.vector.tensor_copy(zT_sb[:], zT[:])
                sT = segps.tile([E, SPB], F32, tag="segps")
                for c in range(HC):
                    nc.tensor.matmul(sT[:], w2[l][:, c, :], zT_sb[:, c, :],
                                     start=(c == 0), stop=(c == HC - 1))
                sT_sb = segsb.tile([E, SPB], F16, tag="sT_sb")
                nc.scalar.copy(sT_sb[:], sT[:])
                ug = segps.tile([SPB, H], F32, tag="segps")
                nc.tensor.matmul(ug[:], sT_sb[:], gw1[l][:], start=True, stop=True)
                ssg = spool.tile([SPB, 1], F32, tag="ssg")
                sqg = segsb.tile([SPB, H], F16, tag="sqg")
                nc.scalar.activation(sqg[:], ug[:], ACTF.Square,
                                     accum_out=ssg[:])
                sdg = spool.tile([SPB, 1], F32, tag="sdg")
                nc.scalar.activation(sdg[:], ssg[:], ACTF.Sqrt,
                                     bias=eps_c[:], scale=1.0 / H)
                rg = spool.tile([SPB, 1], F32, tag="rg")
                nc.vector.reciprocal(rg[:], sdg[:])
                Rg = segsb.tile([SPB, H], F16, tag="Rg")
                nc.vector.tensor_scalar(
                    Rg[:], ug[:], rg[:], 0.0, ALU.mult, ALU.max)
                RgT = segps.tile([128, HC, SPB], F16, tag="segps")
                for c in range(HC):
                    nc.tensor.transpose(RgT[:, c, :], Rg[:, c * 128:(c + 1) * 128], ident[:])
                RgT_sb = segsb.tile([128, HC, SPB], F16, tag="RgT_sb")
                nc.vector.tensor_copy(RgT_sb[:], RgT[:])
                xgT = segps.tile([E, SPB], F32, tag="segps")
                for c in range(HC):
                    nc.tensor.matmul(xgT[:], gw2[l][:, c, :], RgT_sb[:, c, :],
                                     start=(c == 0),
                                     stop=(c == HC - 1 and l > 0))
                if l == 0:
                    nc.tensor.matmul(xgT[:], cw[:], ys[:, blk * SPB:(blk + 1) * SPB],
                                     start=False, stop=True)
                if l < 2:
                    xgT_sb = segsb.tile([E, SPB], F16, tag="xgT_sb")
                    nc.vector.tensor_copy(xgT_sb[:], xgT[:])
                    xgw = segps.tile([SPB, H], F32, tag="segps")
                    nc.tensor.matmul(xgw[:], xgT_sb[:], w1g[l + 1][:],
                                     start=True, stop=True)
                    nc.scalar.copy(xgw_store[:, blk, :], xgw[:])
                else:
                    o_sb = segsb.tile([E, SPB], F32, tag="o_sb")
                    nc.vector.tensor_copy(o_sb[:], xgT[:])
                    nc.sync.dma_start(outT_d[:, blk * SPB:(blk + 1) * SPB], o_sb[:])

            for l in range(3):
                pend_seg = None
                for blk in range(NBLK):
                    xt = xpool.tile([E, MAXBLK * T], F16, tag="xt")
                    nc.sync.dma_start(xt[:], xT_d[:, blk * MAXBLK * T:(blk + 1) * MAXBLK * T])
                    ot = otpool.tile([T, MAXBLK, SPB], F16, tag="ot")
                    nc.sync.dma_start(ot[:], OT_d[blk])
                    if l > 0:
                        og = ogpool.tile([SPB, MAXBLK, T], F16, tag="og")
                        nc.sync.dma_start(og[:], OG_d[blk])
                    z = zpool.tile([SPB, H], F32, tag="z")
                    pend = None
                    for sbi in range(NSB):
                        ss_b = spool.tile([T, SB], F16, tag="ssb")
                        groups = []
                        for gi in range(SB // GRP):
                            a1g = a1pool.tile([T, GRP, H], F32, tag="a1g")
                            for j in range(GRP):
                                ti = sbi * SB + gi * GRP + j
                                nc.tensor.matmul(a1g[:, j, :],
                                                 xt[:, ti * T:(ti + 1) * T],
                                                 w1x[l][:], start=True,
                                                 stop=(l == 0))
                                if l > 0:
                                    nc.tensor.matmul(a1g[:, j, :], og[:, ti, :],
                                                     xgw_store[:, blk, :],
                                                     start=False, stop=True)
                            sq = sqpool.tile([T, GRP, H], F16, tag="sq")
                            nc.scalar.activation(sq[:], a1g[:], ACTF.Square)
                            with nc.allow_low_precision("LN stats tolerate f16"):
                                nc.vector.tensor_reduce(
                                    ss_b[:, gi * GRP:(gi + 1) * GRP], sq[:],
                                    AXL.X, ALU.add)
                            af = afpool.tile([T, GRP, H], F16, tag="af")
                            if gcount[0] % 3 < 2:
                                nc.scalar.activation(af[:], a1g[:], ACTF.Relu)
                            else:
                                nc.vector.tensor_scalar(
                                    af[:], a1g[:], 1.0, 0.0, ALU.mult, ALU.max)
                            gcount[0] += 1
                            groups.append(af)
                        this = (sbi, ss_b, groups)
                        if pend is not None:
                            back_batch(*pend, ot, z)
                        pend = this
                    back_batch(*pend, ot, z)
                    if pend_seg is not None:
                        seg_phase(l, *pend_seg)
                    pend_seg = (blk, z)
                seg_phase(l, *pend_seg)

    nc.compile()
    return nc


def _run(inputs, trace=False):
    geom, shared, percore = _prep_host(inputs)
    nc = _build_program(geom)
    in_maps = []
    for c in range(NCORES):
        m = dict(shared)
        m.update(percore[c])
        in_maps.append(m)
    res = run_bass_kernel_spmd(nc, in_maps, list(range(NCORES)), trace=trace)
    B, E, B_LOC = geom["B"], geom["E"], geom["B_LOC"]
    out = np.empty((B, E), np.float32)
    for c in range(NCORES):
        out[c * B_LOC:(c + 1) * B_LOC] = res.results[c]["outT"].T
    return out, res


def kernel(**inputs):
    out, _ = _run(inputs)
    return out


# revision 13
# speedup vs baseline: 6.6611x; 1.0407x over previous
"""DeeperSet aggregation kernel for 8 Trainium2 NeuronCores.

Strategy: data-parallel over contiguous graph-id ranges (2048 graphs/core).
Segment boundaries are host-known (batch is an input), so segment-sum and
the xg[batch] gather are expressed as matmuls against host-built one-hot
tiles.  LayerNorm (gamma=1, beta=0, biases=0 in this model) reduces to a
per-node positive scale r = 1/sqrt(mean(u^2)+eps) with mean-centering folded
into the weights on the host.  r commutes through ReLU and the segment-sum,
so it is applied to the (half-width) one-hot rows instead of the
activations.  Elementwise work is batched over GRP-tile super-groups to
amortize per-instruction overheads, and emission is software-pipelined
(one stats-batch and one segment-phase of lookahead) so the PE never
stalls and ramps to full clock.
"""

import sys

sys.path.insert(0, "/opt/trn_rl_repo")

import numpy as np

import concourse.bass as bass
import concourse.tile as tile
from concourse import bacc, mybir
from concourse.bass_utils import run_bass_kernel_spmd
from concourse.masks import make_identity

F32 = mybir.dt.float32
F16 = mybir.dt.float16
ALU = mybir.AluOpType
ACTF = mybir.ActivationFunctionType
AXL = mybir.AxisListType

LN_EPS = 1e-5
NCORES = 8
SPB = 128          # segments (graphs) per block
T = 128            # nodes per tile
GRP = 4            # tiles per elementwise super-group
SB = 8             # tiles per stats batch (= 2 groups)


def _center(w, g):
    return ((w - w.mean(axis=1, keepdims=True)) * g[None, :]).astype(np.float32)


def _prep_host(inputs):
    x = np.asarray(inputs["x"], np.float32)
    y = np.asarray(inputs["y"], np.float32)
    batch = np.asarray(inputs["batch"], np.int64)
    N, E = x.shape
    B, YD = y.shape
    H = inputs["l0_lw1"].shape[1]

    for k in ("l0_lb1", "l0_lbt", "l0_lb2", "l0_gb1", "l0_gbt", "l0_gb2",
              "lr_lb1", "lr_lbt", "lr_lb2", "lr_gb1", "lr_gbt", "lr_gb2", "cb"):
        assert np.abs(np.asarray(inputs[k])).max() < 1e-12, f"{k} must be zero"
    for k in ("l0_lg", "l0_gg", "lr_lg", "lr_gg"):
        assert np.abs(np.asarray(inputs[k]) - 1.0).max() < 1e-12, f"{k} must be one"

    B_LOC = B // NCORES
    NBLK = B_LOC // SPB
    edges = np.searchsorted(batch, np.arange(0, B + 1, SPB)).astype(np.int64)
    cnts = np.diff(edges)
    maxblk = int(np.ceil(cnts.max() / T)) if N > 0 else 1
    MAXBLK = max(SB, ((maxblk + SB - 1) // SB) * SB)
    NT = NBLK * MAXBLK          # tiles per core
    NPADC = NT * T              # padded nodes per core

    xT = [np.zeros((E, NPADC), np.float16) for _ in range(NCORES)]
    OT = [np.zeros((NBLK, T, MAXBLK, SPB), np.float16) for _ in range(NCORES)]
    OG = [np.zeros((NBLK, SPB, MAXBLK, T), np.float16) for _ in range(NCORES)]
    ysT = [None] * NCORES
    for c in range(NCORES):
        for k in range(NBLK):
            j = c * NBLK + k
            n0, n1 = int(edges[j]), int(edges[j + 1])
            cnt = n1 - n0
            if cnt == 0:
                continue
            base = k * MAXBLK * T
            xT[c][:, base:base + cnt] = x[n0:n1].T.astype(np.float16)
            a = np.arange(cnt)
            t = a // T
            p = a % T
            g = (batch[n0:n1] - j * SPB).astype(np.int64)
            OT[c][k, p, t, g] = 1.0
            OG[c][k, g, t, p] = 1.0
        ysT[c] = np.ascontiguousarray(y[c * B_LOC:(c + 1) * B_LOC].T).astype(np.float16)

    f16 = lambda w: np.ascontiguousarray(w).astype(np.float16)
    l0_w1f = _center(np.asarray(inputs["l0_lw1"], np.float32), np.asarray(inputs["l0_lg"], np.float32))
    W1X, W1G = [f16(l0_w1f)], [None]
    W2 = [f16(np.asarray(inputs["l0_lw2"], np.float32))]
    GW1 = [f16(_center(np.asarray(inputs["l0_gw1"], np.float32), np.asarray(inputs["l0_gg"], np.float32)))]
    GW2 = [f16(np.asarray(inputs["l0_gw2"], np.float32))]
    for i in range(2):
        w1f = _center(np.asarray(inputs["lr_lw1"][i], np.float32), np.asarray(inputs["lr_lg"][i], np.float32))
        W1X.append(f16(w1f[:E]))
        W1G.append(f16(w1f[E:]))
        W2.append(f16(np.asarray(inputs["lr_lw2"][i], np.float32)))
        GW1.append(f16(_center(np.asarray(inputs["lr_gw1"][i], np.float32), np.asarray(inputs["lr_gg"][i], np.float32))))
        GW2.append(f16(np.asarray(inputs["lr_gw2"][i], np.float32)))
    CW = f16(np.asarray(inputs["cw"], np.float32))

    geom = dict(N=N, E=E, B=B, YD=YD, H=H, B_LOC=B_LOC, NBLK=NBLK,
                MAXBLK=MAXBLK, NT=NT, NPADC=NPADC)
    shared = dict(CW=CW)
    for l in range(3):
        shared[f"W1X{l}"] = W1X[l]
        shared[f"W2_{l}"] = W2[l]
        shared[f"GW1_{l}"] = GW1[l]
        shared[f"GW2_{l}"] = GW2[l]
        if l > 0:
            shared[f"W1G{l}"] = W1G[l]
    percore = [dict(xT=xT[c], OT=OT[c], OG=OG[c], ysT=ysT[c]) for c in range(NCORES)]
    return geom, shared, percore


def _build_program(geom):
    E, H, YD = geom["E"], geom["H"], geom["YD"]
    B_LOC, NBLK, MAXBLK, NT, NPADC = (geom["B_LOC"], geom["NBLK"],
                                      geom["MAXBLK"], geom["NT"], geom["NPADC"])
    HC = H // 128  # H chunks of 128
    NSB = MAXBLK // SB

    nc = bacc.Bacc("TRN2", target_bir_lowering=False, debug=False)

    xT_d = nc.dram_tensor("xT", [E, NPADC], F16, kind="ExternalInput").ap()
    OT_d = nc.dram_tensor("OT", [NBLK, T, MAXBLK, SPB], F16, kind="ExternalInput").ap()
    OG_d = nc.dram_tensor("OG", [NBLK, SPB, MAXBLK, T], F16, kind="ExternalInput").ap()
    ysT_d = nc.dram_tensor("ysT", [YD, B_LOC], F16, kind="ExternalInput").ap()
    CW_d = nc.dram_tensor("CW", [YD, E], F16, kind="ExternalInput").ap()
    W1X_d, W1G_d, W2_d, GW1_d, GW2_d = {}, {}, {}, {}, {}
    for l in range(3):
        W1X_d[l] = nc.dram_tensor(f"W1X{l}", [E, H], F16, kind="ExternalInput").ap()
        W2_d[l] = nc.dram_tensor(f"W2_{l}", [H, E], F16, kind="ExternalInput").ap()
        GW1_d[l] = nc.dram_tensor(f"GW1_{l}", [E, H], F16, kind="ExternalInput").ap()
        GW2_d[l] = nc.dram_tensor(f"GW2_{l}", [H, E], F16, kind="ExternalInput").ap()
        if l > 0:
            W1G_d[l] = nc.dram_tensor(f"W1G{l}", [E, H], F16, kind="ExternalInput").ap()
    outT_d = nc.dram_tensor("outT", [E, B_LOC], F32, kind="ExternalOutput").ap()

    with tile.TileContext(nc) as tc:
        with tc.tile_pool(name="const", bufs=1) as cpool, \
             tc.tile_pool(name="xin", bufs=2) as xpool, \
             tc.tile_pool(name="otin", bufs=2) as otpool, \
             tc.tile_pool(name="ogin", bufs=2) as ogpool, \
             tc.tile_pool(name="rstat", bufs=8) as spool, \
             tc.tile_pool(name="otr", bufs=4) as rpool, \
             tc.tile_pool(name="sqs", bufs=3) as sqpool, \
             tc.tile_pool(name="af", bufs=6) as afpool, \
             tc.tile_pool(name="segsb", bufs=3) as segsb, \
             tc.tile_pool(name="a1ps", bufs=2, space="PSUM") as a1pool, \
             tc.tile_pool(name="zps", bufs=2, space="PSUM") as zpool, \
             tc.tile_pool(name="segps", bufs=2, space="PSUM") as segps:

            # ---- resident constants ----
            def load_const(name, dram_ap, shape, rearr=None):
                tl = cpool.tile(shape, F16, tag=name)
                src = dram_ap if rearr is None else dram_ap.rearrange(rearr, c=HC)
                nc.sync.dma_start(tl[:], src)
                return tl

            w1x = {l: load_const(f"w1x{l}", W1X_d[l], [E, H]) for l in range(3)}
            w1g = {l: load_const(f"w1g{l}", W1G_d[l], [E, H]) for l in (1, 2)}
            gw1 = {l: load_const(f"gw1{l}", GW1_d[l], [E, H]) for l in range(3)}
            # w2 / gw2 as [128, HC, E] chunked stationary operands
            w2 = {l: load_const(f"w2{l}", W2_d[l], [128, HC, E], "(c p) e -> p c e")
                  for l in range(3)}
            gw2 = {l: load_const(f"gw2{l}", GW2_d[l], [128, HC, E], "(c p) e -> p c e")
                   for l in range(3)}
            cw = load_const("cw", CW_d, [YD, E])
            ys = load_const("ys", ysT_d, [YD, B_LOC])
            ident = cpool.tile([128, 128], F16, tag="ident")
            make_identity(nc, ident[:])
            eps_c = cpool.tile([128, 1], F32, tag="eps_c")
            nc.gpsimd.memset(eps_c[:], LN_EPS)
            xgw_store = cpool.tile([128, NBLK, H], F16, tag="xgw")

            gcount = [0]

            def back_batch(sbi, ss_b, groups, ot, z):
                """Stats + one-hot scaling + segment-sum matmuls for a
                completed stats batch (emitted one batch late)."""
                sd = spool.tile([T, SB], F32, tag="sd")
                nc.scalar.activation(sd[:], ss_b[:], ACTF.Sqrt,
                                     bias=eps_c[:], scale=1.0 / H)
                r4 = spool.tile([T, SB], F32, tag="r4")
                nc.vector.reciprocal(r4[:], sd[:])
                for gi, af in enumerate(groups):
                    for j in range(GRP):
                        g = gi * GRP + j
                        ti = sbi * SB + g
                        otr = rpool.tile([T, SPB], F16, tag="otr")
                        nc.vector.tensor_scalar(
                            otr[:], ot[:, ti, :], r4[:, g:g + 1], 0.0,
                            ALU.mult, ALU.max)
                        nc.tensor.matmul(z[:], otr[:], af[:, j, :],
                                         start=(sbi == 0 and g == 0),
                                         stop=(sbi == NSB - 1 and g == SB - 1))

            def seg_phase(l, blk, z):
                """Per-block segment pipeline: z -> s -> global MLP -> xg."""
                z_sb = segsb.tile([SPB, H], F16, tag="z_sb")
                nc.vector.tensor_copy(z_sb[:], z[:])
                zT = segps.tile([128, HC, SPB], F16, tag="segps")
                for c in range(HC):
                    nc.tensor.transpose(zT[:, c, :], z_sb[:, c * 128:(c + 1) * 128], ident[:])
                zT_sb = segsb.tile([128, HC, SPB], F16, tag="zT_sb")
                nc.vector.tensor_copy(zT_sb[:], zT[:])
                sT = segps.tile([E, SPB], F32, tag="segps")
                for c in range(HC):
                    nc.tensor.matmul(sT[:], w2[l][:, c, :], zT_sb[:, c, :],
                                     start=(c == 0), stop=(c == HC - 1))
                sT_sb = segsb.tile([E, SPB], F16, tag="sT_sb")
                nc.scalar.copy(sT_sb[:], sT[:])
                ug = segps.tile([SPB, H], F32, tag="segps")
                nc.tensor.matmul(ug[:], sT_sb[:], gw1[l][:], start=True, stop=True)
                ssg = spool.tile([SPB, 1], F32, tag="ssg")
                sqg = segsb.tile([SPB, H], F16, tag="sqg")
                nc.scalar.activation(sqg[:], ug[:], ACTF.Square,
                                     accum_out=ssg[:])
                sdg = spool.tile([SPB, 1], F32, tag="sdg")
                nc.scalar.activation(sdg[:], ssg[:], ACTF.Sqrt,
                                     bias=eps_c[:], scale=1.0 / H)
                rg = spool.tile([SPB, 1], F32, tag="rg")
                nc.vector.reciprocal(rg[:], sdg[:])
                Rg = segsb.tile([SPB, H], F16, tag="Rg")
                nc.vector.tensor_scalar(
                    Rg[:], ug[:], rg[:], 0.0, ALU.mult, ALU.max)
                RgT = segps.tile([128, HC, SPB], F16, tag="segps")
                for c in range(HC):
                    nc.tensor.transpose(RgT[:, c, :], Rg[:, c * 128:(c + 1) * 128], ident[:])
                RgT_sb = segsb.tile([128, HC, SPB], F16, tag="RgT_sb")
                nc.vector.tensor_copy(RgT_sb[:], RgT[:])
                xgT = segps.tile([E, SPB], F32, tag="segps")
                for c in range(HC):
                    nc.tensor.matmul(xgT[:], gw2[l][:, c, :], RgT_sb[:, c, :],
                                     start=(c == 0),
                                     stop=(c == HC - 1 and l > 0))
                if l == 0:
                    nc.tensor.matmul(xgT[:], cw[:], ys[:, blk * SPB:(blk + 1) * SPB],
                                     start=False, stop=True)
                if l < 2:
                    xgT_sb = segsb.tile([E, SPB], F16, tag="xgT_sb")
                    nc.vector.tensor_copy(xgT_sb[:], xgT[:])
                    xgw = segps.tile([SPB, H], F32, tag="segps")
                    nc.tensor.matmul(xgw[:], xgT_sb[:], w1g[l + 1][:],
                                     start=True, stop=True)
                    nc.scalar.copy(xgw_store[:, blk, :], xgw[:])
                else:
                    o_sb = segsb.tile([E, SPB], F32, tag="o_sb")
                    nc.vector.tensor_copy(o_sb[:], xgT[:])
                    nc.sync.dma_start(outT_d[:, blk * SPB:(blk + 1) * SPB], o_sb[:])

            for l in range(3):
                pend_seg = None
                for blk in range(NBLK):
                    xt = xpool.tile([E, MAXBLK * T], F16, tag="xt")
                    nc.sync.dma_start(xt[:], xT_d[:, blk * MAXBLK * T:(blk + 1) * MAXBLK * T])
                    ot = otpool.tile([T, MAXBLK, SPB], F16, tag="ot")
                    nc.sync.dma_start(ot[:], OT_d[blk])
                    if l > 0:
                        og = ogpool.tile([SPB, MAXBLK, T], F16, tag="og")
                        nc.sync.dma_start(og[:], OG_d[blk])
                    z = zpool.tile([SPB, H], F32, tag="z")
                    pend = None
                    for sbi in range(NSB):
                        ss_b = spool.tile([T, SB], F16, tag="ssb")
                        groups = []
                        for gi in range(SB // GRP):
                            a1g = a1pool.tile([T, GRP, H], F32, tag="a1g")
                            for j in range(GRP):
                                ti = sbi * SB + gi * GRP + j
                                nc.tensor.matmul(a1g[:, j, :],
                                                 xt[:, ti * T:(ti + 1) * T],
                                                 w1x[l][:], start=True,
                                                 stop=(l == 0))
                                if l > 0:
                                    nc.tensor.matmul(a1g[:, j, :], og[:, ti, :],
                                                     xgw_store[:, blk, :],
                                                     start=False, stop=True)
                            sq = sqpool.tile([T, GRP, H], F16, tag="sq")
                            nc.scalar.activation(sq[:], a1g[:], ACTF.Square)
                            with nc.allow_low_precision("LN stats tolerate f16"):
                                nc.vector.tensor_reduce(
                                    ss_b[:, gi * GRP:(gi + 1) * GRP], sq[:],
                                    AXL.X, ALU.add)
                            af = afpool.tile([T, GRP, H], F16, tag="af")
                            if gcount[0] % 5 < 4:
                                nc.scalar.activation(af[:], a1g[:], ACTF.Relu)
                            else:
                                nc.vector.tensor_scalar(
                                    af[:], a1g[:], 1.0, 0.0, ALU.mult, ALU.max)
                            gcount[0] += 1
                            groups.append(af)
                        this = (sbi, ss_b, groups)
                        if pend is not None:
                            back_batch(*pend, ot, z)
                        pend = this
                    back_batch(*pend, ot, z)
                    if pend_seg is not None:
                        seg_phase(l, *pend_seg)
                    pend_seg = (blk, z)
                seg_phase(l, *pend_seg)

    nc.compile()
    return nc


def _run(inputs, trace=False):
    geom, shared, percore = _prep_host(inputs)
    nc = _build_program(geom)
    in_maps = []
    for c in range(NCORES):
        m = dict(shared)
        m.update(percore[c])
        in_maps.append(m)
    res = run_bass_kernel_spmd(nc, in_maps, list(range(NCORES)), trace=trace)
    B, E, B_LOC = geom["B"], geom["E"], geom["B_LOC"]
    out = np.empty((B, E), np.float32)
    for c in range(NCORES):
        out[c * B_LOC:(c + 1) * B_LOC] = res.results[c]["outT"].T
    return out, res


def kernel(**inputs):
    out, _ = _run(inputs)
    return out


# revision 18
# speedup vs baseline: 6.7611x; 1.0150x over previous
"""DeeperSet aggregation kernel for 8 Trainium2 NeuronCores.

Strategy: data-parallel over contiguous graph-id ranges (2048 graphs/core).
Segment boundaries are host-known (batch is an input), so segment-sum and
the xg[batch] gather are expressed as matmuls against host-built one-hot
tiles.  LayerNorm (gamma=1, beta=0, biases=0 in this model) reduces to a
per-node positive scale r = 1/sqrt(mean(u^2)+eps) with mean-centering folded
into the weights on the host.  r commutes through ReLU and the segment-sum,
so it is applied to the (half-width) one-hot rows instead of the
activations.  Elementwise work is batched over GRP-tile super-groups to
amortize per-instruction overheads, and emission is software-pipelined
(one stats-batch and one segment-phase of lookahead) so the PE never
stalls and ramps to full clock.
"""

import sys

sys.path.insert(0, "/opt/trn_rl_repo")

import numpy as np

import concourse.bass as bass
import concourse.tile as tile
from concourse import bacc, mybir
from concourse.bass_utils import run_bass_kernel_spmd
from concourse.masks import make_identity

F32 = mybir.dt.float32
F16 = mybir.dt.float16
ALU = mybir.AluOpType
ACTF = mybir.ActivationFunctionType
AXL = mybir.AxisListType

LN_EPS = 1e-5
NCORES = 8
SPB = 128          # segments (graphs) per block
T = 128            # nodes per tile
GRP = 4            # tiles per elementwise super-group
SB = 8             # tiles per stats batch (= 2 groups)


def _center(w, g):
    return ((w - w.mean(axis=1, keepdims=True)) * g[None, :]).astype(np.float32)


def _prep_host(inputs):
    x = np.asarray(inputs["x"], np.float32)
    y = np.asarray(inputs["y"], np.float32)
    batch = np.asarray(inputs["batch"], np.int64)
    N, E = x.shape
    B, YD = y.shape
    H = inputs["l0_lw1"].shape[1]

    for k in ("l0_lb1", "l0_lbt", "l0_lb2", "l0_gb1", "l0_gbt", "l0_gb2",
              "lr_lb1", "lr_lbt", "lr_lb2", "lr_gb1", "lr_gbt", "lr_gb2", "cb"):
        assert np.abs(np.asarray(inputs[k])).max() < 1e-12, f"{k} must be zero"
    for k in ("l0_lg", "l0_gg", "lr_lg", "lr_gg"):
        assert np.abs(np.asarray(inputs[k]) - 1.0).max() < 1e-12, f"{k} must be one"

    B_LOC = B // NCORES
    NBLK = B_LOC // SPB
    edges = np.searchsorted(batch, np.arange(0, B + 1, SPB)).astype(np.int64)
    cnts = np.diff(edges)
    maxblk = int(np.ceil(cnts.max() / T)) if N > 0 else 1
    MAXBLK = max(SB, ((maxblk + SB - 1) // SB) * SB)
    NT = NBLK * MAXBLK          # tiles per core
    NPADC = NT * T              # padded nodes per core

    xT = [np.zeros((E, NPADC), np.float16) for _ in range(NCORES)]
    OT = [np.zeros((NBLK, T, MAXBLK, SPB), np.float16) for _ in range(NCORES)]
    OG = [np.zeros((NBLK, SPB, MAXBLK, T), np.float16) for _ in range(NCORES)]
    ysT = [None] * NCORES
    for c in range(NCORES):
        for k in range(NBLK):
            j = c * NBLK + k
            n0, n1 = int(edges[j]), int(edges[j + 1])
            cnt = n1 - n0
            if cnt == 0:
                continue
            base = k * MAXBLK * T
            xT[c][:, base:base + cnt] = x[n0:n1].T.astype(np.float16)
            a = np.arange(cnt)
            t = a // T
            p = a % T
            g = (batch[n0:n1] - j * SPB).astype(np.int64)
            OT[c][k, p, t, g] = 1.0
            OG[c][k, g, t, p] = 1.0
        ysT[c] = np.ascontiguousarray(y[c * B_LOC:(c + 1) * B_LOC].T).astype(np.float16)

    f16 = lambda w: np.ascontiguousarray(w).astype(np.float16)
    l0_w1f = _center(np.asarray(inputs["l0_lw1"], np.float32), np.asarray(inputs["l0_lg"], np.float32))
    W1X, W1G = [f16(l0_w1f)], [None]
    W2 = [f16(np.asarray(inputs["l0_lw2"], np.float32))]
    GW1 = [f16(_center(np.asarray(inputs["l0_gw1"], np.float32), np.asarray(inputs["l0_gg"], np.float32)))]
    GW2 = [f16(np.asarray(inputs["l0_gw2"], np.float32))]
    for i in range(2):
        w1f = _center(np.asarray(inputs["lr_lw1"][i], np.float32), np.asarray(inputs["lr_lg"][i], np.float32))
        W1X.append(f16(w1f[:E]))
        W1G.append(f16(w1f[E:]))
        W2.append(f16(np.asarray(inputs["lr_lw2"][i], np.float32)))
        GW1.append(f16(_center(np.asarray(inputs["lr_gw1"][i], np.float32), np.asarray(inputs["lr_gg"][i], np.float32))))
        GW2.append(f16(np.asarray(inputs["lr_gw2"][i], np.float32)))
    CW = f16(np.asarray(inputs["cw"], np.float32))

    geom = dict(N=N, E=E, B=B, YD=YD, H=H, B_LOC=B_LOC, NBLK=NBLK,
                MAXBLK=MAXBLK, NT=NT, NPADC=NPADC)
    shared = dict(CW=CW)
    for l in range(3):
        shared[f"W1X{l}"] = W1X[l]
        shared[f"W2_{l}"] = W2[l]
        shared[f"GW1_{l}"] = GW1[l]
        shared[f"GW2_{l}"] = GW2[l]
        if l > 0:
            shared[f"W1G{l}"] = W1G[l]
    percore = [dict(xT=xT[c], OT=OT[c], OG=OG[c], ysT=ysT[c]) for c in range(NCORES)]
    return geom, shared, percore


def _build_program(geom):
    E, H, YD = geom["E"], geom["H"], geom["YD"]
    B_LOC, NBLK, MAXBLK, NT, NPADC = (geom["B_LOC"], geom["NBLK"],
                                      geom["MAXBLK"], geom["NT"], geom["NPADC"])
    HC = H // 128  # H chunks of 128
    NSB = MAXBLK // SB

    nc = bacc.Bacc("TRN2", target_bir_lowering=False, debug=False)

    xT_d = nc.dram_tensor("xT", [E, NPADC], F16, kind="ExternalInput").ap()
    OT_d = nc.dram_tensor("OT", [NBLK, T, MAXBLK, SPB], F16, kind="ExternalInput").ap()
    OG_d = nc.dram_tensor("OG", [NBLK, SPB, MAXBLK, T], F16, kind="ExternalInput").ap()
    ysT_d = nc.dram_tensor("ysT", [YD, B_LOC], F16, kind="ExternalInput").ap()
    CW_d = nc.dram_tensor("CW", [YD, E], F16, kind="ExternalInput").ap()
    W1X_d, W1G_d, W2_d, GW1_d, GW2_d = {}, {}, {}, {}, {}
    for l in range(3):
        W1X_d[l] = nc.dram_tensor(f"W1X{l}", [E, H], F16, kind="ExternalInput").ap()
        W2_d[l] = nc.dram_tensor(f"W2_{l}", [H, E], F16, kind="ExternalInput").ap()
        GW1_d[l] = nc.dram_tensor(f"GW1_{l}", [E, H], F16, kind="ExternalInput").ap()
        GW2_d[l] = nc.dram_tensor(f"GW2_{l}", [H, E], F16, kind="ExternalInput").ap()
        if l > 0:
            W1G_d[l] = nc.dram_tensor(f"W1G{l}", [E, H], F16, kind="ExternalInput").ap()
    outT_d = nc.dram_tensor("outT", [E, B_LOC], F32, kind="ExternalOutput").ap()

    with tile.TileContext(nc) as tc:
        with tc.tile_pool(name="const", bufs=1) as cpool, \
             tc.tile_pool(name="xin", bufs=2) as xpool, \
             tc.tile_pool(name="otin", bufs=2) as otpool, \
             tc.tile_pool(name="ogin", bufs=2) as ogpool, \
             tc.tile_pool(name="rstat", bufs=8) as spool, \
             tc.tile_pool(name="otr", bufs=4) as rpool, \
             tc.tile_pool(name="sqs", bufs=3) as sqpool, \
             tc.tile_pool(name="af", bufs=6) as afpool, \
             tc.tile_pool(name="segsb", bufs=3) as segsb, \
             tc.tile_pool(name="a1ps", bufs=2, space="PSUM") as a1pool, \
             tc.tile_pool(name="zps", bufs=2, space="PSUM") as zpool, \
             tc.tile_pool(name="segps", bufs=2, space="PSUM") as segps:

            # ---- resident constants ----
            def load_const(name, dram_ap, shape, rearr=None):
                tl = cpool.tile(shape, F16, tag=name)
                src = dram_ap if rearr is None else dram_ap.rearrange(rearr, c=HC)
                nc.sync.dma_start(tl[:], src)
                return tl

            w1x = {l: load_const(f"w1x{l}", W1X_d[l], [E, H]) for l in range(3)}
            w1g = {l: load_const(f"w1g{l}", W1G_d[l], [E, H]) for l in (1, 2)}
            gw1 = {l: load_const(f"gw1{l}", GW1_d[l], [E, H]) for l in range(3)}
            # w2 / gw2 as [128, HC, E] chunked stationary operands
            w2 = {l: load_const(f"w2{l}", W2_d[l], [128, HC, E], "(c p) e -> p c e")
                  for l in range(3)}
            gw2 = {l: load_const(f"gw2{l}", GW2_d[l], [128, HC, E], "(c p) e -> p c e")
                   for l in range(3)}
            cw = load_const("cw", CW_d, [YD, E])
            ys = load_const("ys", ysT_d, [YD, B_LOC])
            ident = cpool.tile([128, 128], F16, tag="ident")
            make_identity(nc, ident[:])
            eps_c = cpool.tile([128, 1], F32, tag="eps_c")
            nc.gpsimd.memset(eps_c[:], LN_EPS)
            xgw_store = cpool.tile([128, NBLK, H], F16, tag="xgw")

            gcount = [0]

            def back_batch(sbi, ss_b, groups, ot, z):
                """Stats + one-hot scaling + segment-sum matmuls for a
                completed stats batch (emitted one batch late)."""
                sd = spool.tile([T, SB], F32, tag="sd")
                nc.scalar.activation(sd[:], ss_b[:], ACTF.Sqrt,
                                     bias=eps_c[:], scale=1.0 / H)
                r4 = spool.tile([T, SB], F32, tag="r4")
                nc.vector.reciprocal(r4[:], sd[:])
                for gi, af in enumerate(groups):
                    for j in range(GRP):
                        g = gi * GRP + j
                        ti = sbi * SB + g
                        otr = rpool.tile([T, SPB], F16, tag="otr")
                        nc.vector.tensor_scalar(
                            otr[:], ot[:, ti, :], r4[:, g:g + 1], 0.0,
                            ALU.mult, ALU.max)
                        nc.tensor.matmul(z[:], otr[:], af[:, j, :],
                                         start=(sbi == 0 and g == 0),
                                         stop=(sbi == NSB - 1 and g == SB - 1))

            def seg_phase(l, blk, z):
                """Per-block segment pipeline: z -> s -> global MLP -> xg.
                Generator: yields between stages so the driver can interleave
                them with the next block's batches (keeps the PE queue free
                of head-of-line waits on the serial cross-engine chain)."""
                z_sb = segsb.tile([SPB, H], F16, tag="z_sb")
                nc.vector.tensor_copy(z_sb[:], z[:])
                zT = segps.tile([128, HC, SPB], F16, tag="segps")
                for c in range(HC):
                    nc.tensor.transpose(zT[:, c, :], z_sb[:, c * 128:(c + 1) * 128], ident[:])
                zT_sb = segsb.tile([128, HC, SPB], F16, tag="zT_sb")
                nc.vector.tensor_copy(zT_sb[:], zT[:])
                yield
                sT = segps.tile([E, SPB], F32, tag="segps")
                for c in range(HC):
                    nc.tensor.matmul(sT[:], w2[l][:, c, :], zT_sb[:, c, :],
                                     start=(c == 0), stop=(c == HC - 1))
                sT_sb = segsb.tile([E, SPB], F16, tag="sT_sb")
                nc.scalar.copy(sT_sb[:], sT[:])
                yield
                ug = segps.tile([SPB, H], F32, tag="segps")
                nc.tensor.matmul(ug[:], sT_sb[:], gw1[l][:], start=True, stop=True)
                ssg = spool.tile([SPB, 1], F32, tag="ssg")
                sqg = segsb.tile([SPB, H], F16, tag="sqg")
                nc.scalar.activation(sqg[:], ug[:], ACTF.Square,
                                     accum_out=ssg[:])
                sdg = spool.tile([SPB, 1], F32, tag="sdg")
                nc.scalar.activation(sdg[:], ssg[:], ACTF.Sqrt,
                                     bias=eps_c[:], scale=1.0 / H)
                rg = spool.tile([SPB, 1], F32, tag="rg")
                nc.vector.reciprocal(rg[:], sdg[:])
                Rg = segsb.tile([SPB, H], F16, tag="Rg")
                nc.vector.tensor_scalar(
                    Rg[:], ug[:], rg[:], 0.0, ALU.mult, ALU.max)
                yield
                RgT = segps.tile([128, HC, SPB], F16, tag="segps")
                for c in range(HC):
                    nc.tensor.transpose(RgT[:, c, :], Rg[:, c * 128:(c + 1) * 128], ident[:])
                RgT_sb = segsb.tile([128, HC, SPB], F16, tag="RgT_sb")
                nc.vector.tensor_copy(RgT_sb[:], RgT[:])
                yield
                xgT = segps.tile([E, SPB], F32, tag="segps")
                for c in range(HC):
                    nc.tensor.matmul(xgT[:], gw2[l][:, c, :], RgT_sb[:, c, :],
                                     start=(c == 0),
                                     stop=(c == HC - 1 and l > 0))
                if l == 0:
                    nc.tensor.matmul(xgT[:], cw[:], ys[:, blk * SPB:(blk + 1) * SPB],
                                     start=False, stop=True)
                if l < 2:
                    xgT_sb = segsb.tile([E, SPB], F16, tag="xgT_sb")
                    nc.vector.tensor_copy(xgT_sb[:], xgT[:])
                    yield
                    xgw = segps.tile([SPB, H], F32, tag="segps")
                    nc.tensor.matmul(xgw[:], xgT_sb[:], w1g[l + 1][:],
                                     start=True, stop=True)
                    nc.scalar.copy(xgw_store[:, blk, :], xgw[:])
                else:
                    o_sb = segsb.tile([E, SPB], F32, tag="o_sb")
                    nc.vector.tensor_copy(o_sb[:], xgT[:])
                    nc.sync.dma_start(outT_d[:, blk * SPB:(blk + 1) * SPB], o_sb[:])

            def advance(gen):
                if gen is None:
                    return None
                try:
                    next(gen)
                    return gen
                except StopIteration:
                    return None

            pend_seg = None
            for l in range(3):
                for blk in range(NBLK):
                    xt = xpool.tile([E, MAXBLK * T], F16, tag="xt")
                    nc.sync.dma_start(xt[:], xT_d[:, blk * MAXBLK * T:(blk + 1) * MAXBLK * T])
                    ot = otpool.tile([T, MAXBLK, SPB], F16, tag="ot")
                    nc.sync.dma_start(ot[:], OT_d[blk])
                    if l > 0:
                        og = ogpool.tile([SPB, MAXBLK, T], F16, tag="og")
                        nc.sync.dma_start(og[:], OG_d[blk])
                    z = zpool.tile([SPB, H], F32, tag="z")
                    pend = None
                    for sbi in range(NSB):
                        ss_b = spool.tile([T, SB], F16, tag="ssb")
                        groups = []
                        for gi in range(SB // GRP):
                            a1g = a1pool.tile([T, GRP, H], F32, tag="a1g")
                            for j in range(GRP):
                                ti = sbi * SB + gi * GRP + j
                                nc.tensor.matmul(a1g[:, j, :],
                                                 xt[:, ti * T:(ti + 1) * T],
                                                 w1x[l][:], start=True,
                                                 stop=(l == 0))
                                if l > 0:
                                    nc.tensor.matmul(a1g[:, j, :], og[:, ti, :],
                                                     xgw_store[:, blk, :],
                                                     start=False, stop=True)
                            sq = sqpool.tile([T, GRP, H], F16, tag="sq")
                            nc.scalar.activation(sq[:], a1g[:], ACTF.Square)
                            with nc.allow_low_precision("LN stats tolerate f16"):
                                nc.vector.tensor_reduce(
                                    ss_b[:, gi * GRP:(gi + 1) * GRP], sq[:],
                                    AXL.X, ALU.add)
                            af = afpool.tile([T, GRP, H], F16, tag="af")
                            if gcount[0] % 5 < 4:
                                nc.scalar.activation(af[:], a1g[:], ACTF.Relu)
                            else:
                                nc.vector.tensor_scalar(
                                    af[:], a1g[:], 1.0, 0.0, ALU.mult, ALU.max)
                            gcount[0] += 1
                            groups.append(af)
                        this = (sbi, ss_b, groups)
                        if pend is not None:
                            back_batch(*pend, ot, z)
                        pend_seg = advance(pend_seg)
                        pend = this
                    back_batch(*pend, ot, z)
                    # finish the previous block's segment stages before its z
                    # PSUM slot can be recycled (zpool bufs=2), then hand off
                    while pend_seg is not None:
                        pend_seg = advance(pend_seg)
                    pend_seg = seg_phase(l, blk, z)
            while pend_seg is not None:
                pend_seg = advance(pend_seg)

    nc.compile()
    return nc


def _run(inputs, trace=False):
    geom, shared, percore = _prep_host(inputs)
    nc = _build_program(geom)
    in_maps = []
    for c in range(NCORES):
        m = dict(shared)
        m.update(percore[c])
        in_maps.append(m)
    res = run_bass_kernel_spmd(nc, in_maps, list(range(NCORES)), trace=trace)
    B, E, B_LOC = geom["B"], geom["E"], geom["B_LOC"]
    out = np.empty((B, E), np.float32)
    for c in range(NCORES):
        out[c * B_LOC:(c + 1) * B_LOC] = res.results[c]["outT"].T
    return out, res


def kernel(**inputs):
    out, _ = _run(inputs)
    return out
